# revision 20
# baseline (speedup 1.0000x reference)
"""Segment-mean (sorted index) Trainium2 Bass kernel.

Algorithm (per core, data-parallel over elements, 8 cores):
  - Core gets a contiguous shard of E elements laid out as 128 partitions x
    (E/128) contiguous elements; each partition holds RPP rows of 256 elements.
  - Structure of the input (verified cheaply in kernel()): index is sorted and
    the row-head sequence h[r] = idx[256*r] advances by 0 or 1 between
    consecutive rows, so each 256-row spans at most 2 segments.
  - Phase A (streaming, memory bound): per row r compute
        S[r] = sum(x)                      (row sum)
        T[r] = sum((idx - h[r]) * x)       (tail part: elements of bin h[r]+1)
        I[r] = sum(idx)  (int32, exact)    -> tail count C[r] = I[r] - 256*h[r]
    head_sum = S - T, head_cnt = 256 - C.
  - Phase B: rows with equal h form runs; a segmented scan (reset at run
    start, previous run's tail injected at the run start) yields at the last
    row of each run the complete per-bin (sum, count) for bin h.  A
    per-partition gpsimd local_scatter places each record at the statically
    aligned slot s = h - base0 - K*p + OFS of a 256-wide window (alignment
    verified on the host).  Partition-seam corrections and the core-tail
    record ride as two extra scatter records per partition.
  - Assembly (race-free): windows (zero everywhere no record landed) are
    DMA'd to DRAM with row pitch PITCH and zero guard rows; the statically
    shifted views m in [m_lo, m_hi] are added (overlap cells are exact
    zeros), producing disjoint K-wide strips; one indirect DMA writes the
    128 disjoint strips at element offset base0 + K*p into a [2*SLAB] slab.
  - AllReduce(add) over slabs across 8 cores, then mean = sum / max(cnt, 1).
"""

import sys

sys.path.insert(0, "/opt/trn_rl_repo")

import numpy as np

from concourse import bacc, bass, mybir
from concourse import tile
from concourse.bass_utils import run_bass_kernel_spmd

F32 = mybir.dt.float32
I32 = mybir.dt.int32
I16 = mybir.dt.int16
U16 = mybir.dt.uint16

AX = mybir.AxisListType.X
OP = mybir.AluOpType

N_CORES = 8
P = 128
ROW = 256
NSEG = 100000
SLAB = 100224  # 128 * 783 >= NSEG + K*P slack
WIN = 256  # window cells per partition (f32)


def build_nc(
    epc: int,
    n_chunks: int,
    idx64: bool,
    K: int = 98,
    OFS: int = 80,
    slab: int = SLAB,
    nseg: int = NSEG,
):
    """Build the per-core bass program. epc = P * rpp * ROW elements."""
    assert epc % (P * ROW) == 0
    epp = epc // P
    rpp = epp // ROW
    assert rpp % n_chunks == 0
    cr = rpp // n_chunks
    cf = cr * ROW
    assert slab % P == 0

    # fold geometry
    m_lo = -((WIN - OFS - 1) // K)
    m_hi = (OFS + K - 1) // K
    pitch = max(OFS - m_lo * K + K, WIN + (m_hi * K - OFS))
    pitch = ((pitch + 31) // 32) * 32
    mpad = max(-m_lo, m_hi) + 1
    wf_rows = ((P + 2 * mpad + 3) // 4) * 4  # x4 so wf_rows*pitch % P == 0
    assert K * P <= slab - 64

    nc = bacc.Bacc("TRN2", target_bir_lowering=False, debug=False, num_devices=N_CORES)

    if idx64:
        idx_ext = nc.declare_dram_parameter("idx", [epc, 2], I32, isOutput=False)
    else:
        idx_ext = nc.declare_dram_parameter("idx", [epc], I32, isOutput=False)
    x_ext = nc.declare_dram_parameter("x", [epc], F32, isOutput=False)
    out_ext = nc.declare_dram_parameter("out", [nseg], F32, isOutput=True)

    x_v = x_ext.ap().rearrange("(p e) -> p e", p=P)
    if idx64:
        i_v = idx_ext.ap().rearrange("(p e) w -> p e w", p=P)
    else:
        i_v = idx_ext.ap().rearrange("(p e) -> p e", p=P)

    with tile.TileContext(nc) as tc:
        with (
            tc.tile_pool(name="xs", bufs=2) as xpool,
            tc.tile_pool(name="is_", bufs=2) as ipool,
            tc.tile_pool(name="wk", bufs=2) as wkpool,
            tc.tile_pool(name="pers", bufs=1) as pp,
            tc.tile_pool(name="dram", bufs=1, space="DRAM") as dp,
        ):
            slab_t = dp.tile([2 * slab], F32, tag="slab")
            ar_t = dp.tile([2 * slab], F32, tag="ar", addr_space="Shared")
            mean_t = dp.tile([slab], F32, tag="mean")
            b1_t = dp.tile([P + 1, 1], I32, tag="b1")
            b2_t = dp.tile([P + 1, 5], F32, tag="b2")
            wfA_t = dp.tile([wf_rows, pitch], F32, tag="wfA")
            wfC_t = dp.tile([wf_rows, pitch], F32, tag="wfC")

            H = pp.tile([P, rpp], I32, tag="H")  # row heads
            TS = pp.tile([P, rpp], F32, tag="TS")  # tail sums
            RS = pp.tile([P, rpp], F32, tag="RS")  # row sums
            TCf = pp.tile([P, rpp], F32, tag="TCf")  # tail counts (exact, <=256)

            # K*p per-partition constant (gpsimd iota; standard library)
            Kp = pp.tile([P, 1], I32, tag="Kp")
            nc.gpsimd.iota(Kp[:], pattern=[[0, 1]], base=0, channel_multiplier=K)

            # ---------------- Phase A: stream chunks ----------------
            for c in range(n_chunks):
                cs = slice(c * cr, (c + 1) * cr)
                xt = xpool.tile([P, cf], F32, tag="x")
                it = ipool.tile([P, cf], I32, tag="i")
                nc.sync.dma_start(out=xt[:], in_=x_v[:, c * cf : (c + 1) * cf])
                if idx64:
                    nc.sync.dma_start(
                        out=it[:],
                        in_=i_v[:, c * cf : (c + 1) * cf, 0:1].squeeze(axis=2),
                    )
                else:
                    nc.sync.dma_start(out=it[:], in_=i_v[:, c * cf : (c + 1) * cf])

                i3 = it[:].rearrange("p (r e) -> p r e", e=ROW)
                x3 = xt[:].rearrange("p (r e) -> p r e", e=ROW)

                nc.vector.tensor_copy(out=H[:, cs], in_=i3[:, :, 0:1].squeeze(axis=2))
                hb = H[:, cs].unsqueeze(2).to_broadcast([P, cr, ROW])

                dt_ = wkpool.tile([P, cf], F32, tag="d")
                dx = wkpool.tile([P, cf], F32, tag="dx")
                d3 = dt_[:].rearrange("p (r e) -> p r e", e=ROW)
                dx3 = dx[:].rearrange("p (r e) -> p r e", e=ROW)
                nc.vector.tensor_tensor(out=d3, in0=i3, in1=hb, op=OP.subtract)
                nc.vector.tensor_tensor(out=dx3, in0=d3, in1=x3, op=OP.mult)

                nc.vector.tensor_reduce(out=TS[:, cs], in_=dx3, axis=AX, op=OP.add)
                nc.vector.tensor_reduce(out=RS[:, cs], in_=x3, axis=AX, op=OP.add)
                nc.vector.tensor_reduce(out=TCf[:, cs], in_=d3, axis=AX, op=OP.add)

            # ---------------- Phase B ----------------
            # run flags
            same = pp.tile([P, rpp], F32, tag="same")
            nots = pp.tile([P, rpp], F32, tag="nots")
            nc.vector.memset(same[:, 0:1], 0)
            nc.vector.memset(nots[:, 0:1], 0)
            nc.vector.tensor_tensor(
                out=same[:, 1:], in0=H[:, 1:], in1=H[:, :-1], op=OP.is_equal
            )
            nc.vector.tensor_tensor(
                out=nots[:, 1:], in0=H[:, 1:], in1=H[:, :-1], op=OP.not_equal
            )

            # dataA = (RS - TS) + nots*TS_prev ; dataC = (256 - TCf) + nots*TCf_prev
            dataA = pp.tile([P, rpp], F32, tag="dataA")
            dataC = pp.tile([P, rpp], F32, tag="dataC")
            inj = pp.tile([P, rpp], F32, tag="inj")
            nc.vector.tensor_tensor(out=dataA[:], in0=RS[:], in1=TS[:], op=OP.subtract)
            nc.vector.memset(inj[:, 0:1], 0)
            nc.vector.tensor_tensor(
                out=inj[:, 1:], in0=nots[:, 1:], in1=TS[:, :-1], op=OP.mult
            )
            nc.vector.tensor_tensor(out=dataA[:], in0=dataA[:], in1=inj[:], op=OP.add)
            nc.vector.tensor_scalar(
                out=dataC[:], in0=TCf[:], scalar1=-1.0, scalar2=float(ROW),
                op0=OP.mult, op1=OP.add,
            )
            nc.vector.tensor_tensor(
                out=inj[:, 1:], in0=nots[:, 1:], in1=TCf[:, :-1], op=OP.mult
            )
            nc.vector.memset(inj[:, 0:1], 0)
            nc.vector.tensor_tensor(out=dataC[:], in0=dataC[:], in1=inj[:], op=OP.add)

            # segmented scans
            scanA = pp.tile([P, rpp], F32, tag="scanA")
            scanC = pp.tile([P, rpp], F32, tag="scanC")
            nc.vector.tensor_tensor_scan(
                out=scanA[:], data0=same[:], data1=dataA[:], initial=0.0,
                op0=OP.mult, op1=OP.add,
            )
            nc.vector.tensor_tensor_scan(
                out=scanC[:], data0=same[:], data1=dataC[:], initial=0.0,
                op0=OP.mult, op1=OP.add,
            )

            # seam bounce 1: Hnf[p] = H[p+1, 0] (sentinel -1 at p=127)
            Hnf = pp.tile([P, 1], I32, tag="Hnf")
            sent1 = pp.tile([1, 1], I32, tag="sent1")
            nc.vector.memset(sent1[:], -1)
            nc.sync.dma_start(out=b1_t[0:P, :], in_=H[:, 0:1])
            nc.sync.dma_start(out=b1_t[P : P + 1, :], in_=sent1[:])
            nc.sync.dma_start(out=Hnf[:], in_=b1_t[1 : P + 1, :])

            # base0 broadcast from b1_t[0]
            base0 = pp.tile([P, 1], I32, tag="base0")
            nc.sync.dma_start(
                out=base0[:], in_=b1_t[0:1, 0:1].to_broadcast([P, 1])
            )

            # last-of-run mask with partition-seam suppression at col 127
            lastm = pp.tile([P, rpp], F32, tag="lastm")
            nc.vector.tensor_tensor(
                out=lastm[:, : rpp - 1], in0=H[:, : rpp - 1], in1=H[:, 1:],
                op=OP.not_equal,
            )
            nc.vector.tensor_tensor(
                out=lastm[:, rpp - 1 : rpp], in0=H[:, rpp - 1 : rpp], in1=Hnf[:],
                op=OP.not_equal,
            )

            # seam bounce 2: prev partition's col-127 of [H, scanA, scanC, TS, TCf]
            stage = pp.tile([P, 5], F32, tag="stage")
            nc.vector.tensor_copy(out=stage[:, 0:1], in_=H[:, rpp - 1 : rpp])
            nc.vector.tensor_copy(out=stage[:, 1:2], in_=scanA[:, rpp - 1 : rpp])
            nc.vector.tensor_copy(out=stage[:, 2:3], in_=scanC[:, rpp - 1 : rpp])
            nc.vector.tensor_copy(out=stage[:, 3:4], in_=TS[:, rpp - 1 : rpp])
            nc.vector.tensor_copy(out=stage[:, 4:5], in_=TCf[:, rpp - 1 : rpp])
            prev = pp.tile([P, 5], F32, tag="prev")
            sent5 = pp.tile([1, 5], F32, tag="sent5")
            nc.vector.memset(sent5[:], -999.0)
            nc.sync.dma_start(out=b2_t[1 : P + 1, :], in_=stage[:])
            nc.sync.dma_start(out=b2_t[0:1, :], in_=sent5[:])
            nc.sync.dma_start(out=prev[:], in_=b2_t[0:P, :])

            # corrections: corr = cont*prev_scanA + tailc*prev_TS (cnt analogous)
            h0f = pp.tile([P, 1], F32, tag="h0f")
            cont = pp.tile([P, 1], F32, tag="cont")
            tailc = pp.tile([P, 1], F32, tag="tailc")
            tmp1 = pp.tile([P, 1], F32, tag="tmp1")
            corrB = pp.tile([P, 2], F32, tag="corrB")  # [corr, TS_last]
            corrBC = pp.tile([P, 2], F32, tag="corrBC")  # [corrC, TCf_last]
            nc.vector.tensor_copy(out=h0f[:], in_=H[:, 0:1])
            nc.vector.tensor_tensor(
                out=cont[:], in0=h0f[:], in1=prev[:, 0:1], op=OP.is_equal
            )
            nc.vector.tensor_scalar(
                out=tmp1[:], in0=prev[:, 0:1], scalar1=1.0, scalar2=None, op0=OP.add
            )
            nc.vector.tensor_tensor(
                out=tailc[:], in0=h0f[:], in1=tmp1[:], op=OP.is_equal
            )
            nc.vector.tensor_tensor(
                out=corrB[:, 0:1], in0=cont[:], in1=prev[:, 1:2], op=OP.mult
            )
            nc.vector.tensor_tensor(out=tmp1[:], in0=tailc[:], in1=prev[:, 3:4], op=OP.mult)
            nc.vector.tensor_tensor(
                out=corrB[:, 0:1], in0=corrB[:, 0:1], in1=tmp1[:], op=OP.add
            )
            nc.vector.tensor_tensor(
                out=corrBC[:, 0:1], in0=cont[:], in1=prev[:, 2:3], op=OP.mult
            )
            nc.vector.tensor_tensor(out=tmp1[:], in0=tailc[:], in1=prev[:, 4:5], op=OP.mult)
            nc.vector.tensor_tensor(
                out=corrBC[:, 0:1], in0=corrBC[:, 0:1], in1=tmp1[:], op=OP.add
            )
            # second slot: core-tail values (valid at p=127 only, masked later)
            nc.vector.tensor_copy(out=corrB[:, 1:2], in_=TS[:, rpp - 1 : rpp])
            nc.vector.tensor_copy(out=corrBC[:, 1:2], in_=TCf[:, rpp - 1 : rpp])

            # aligned slots: slot = H - base0 - K*p + OFS
            slotf = pp.tile([P, rpp], F32, tag="slotf")
            sbase = pp.tile([P, 1], I32, tag="sbase")
            nc.vector.tensor_tensor(out=sbase[:], in0=base0[:], in1=Kp[:], op=OP.add)
            nc.vector.tensor_scalar(
                out=sbase[:], in0=sbase[:], scalar1=-OFS, scalar2=None, op0=OP.add
            )
            nc.vector.tensor_tensor(
                out=slotf[:], in0=H[:],
                in1=sbase[:].to_broadcast([P, rpp]), op=OP.subtract,
            )

            # idxA = lastm ? slot : -1 ; u16-pair indices
            idxAf = pp.tile([P, rpp], F32, tag="idxAf")
            nc.vector.tensor_scalar(
                out=idxAf[:], in0=slotf[:], scalar1=1.0, scalar2=None, op0=OP.add
            )
            nc.vector.tensor_tensor(out=idxAf[:], in0=idxAf[:], in1=lastm[:], op=OP.mult)
            nc.vector.tensor_scalar(
                out=idxAf[:], in0=idxAf[:], scalar1=-1.0, scalar2=None, op0=OP.add
            )
            pidxf = pp.tile([P, 2 * rpp], F32, tag="pidxf")
            p3 = pidxf[:].rearrange("p (r w) -> p r w", w=2)
            t2 = pp.tile([P, rpp], F32, tag="t2")
            nc.vector.tensor_scalar(
                out=t2[:], in0=idxAf[:], scalar1=2.0, scalar2=None, op0=OP.mult
            )
            nc.vector.tensor_copy(out=p3[:, :, 0:1].squeeze(axis=2), in_=t2[:])
            nc.vector.tensor_scalar(
                out=t2[:], in0=t2[:], scalar1=1.0, scalar2=None, op0=OP.add
            )
            nc.vector.tensor_copy(out=p3[:, :, 1:2].squeeze(axis=2), in_=t2[:])
            pidx16 = pp.tile([P, 2 * rpp], I16, tag="pidx16")
            nc.vector.tensor_copy(out=pidx16[:], in_=pidxf[:])

            # extra records: [corr at slot(H[p,0]) (all p), core-tail at
            # slot(H[p,last])+1 (p=127 only, via Hnf sentinel mask)]
            vmask = pp.tile([P, 1], F32, tag="vmask")
            nc.vector.tensor_scalar(
                out=vmask[:], in0=Hnf[:], scalar1=-1, scalar2=None, op0=OP.is_equal
            )
            pidxTf = pp.tile([P, 4], F32, tag="pidxTf")
            u2 = pp.tile([P, 1], F32, tag="u2")
            nc.vector.tensor_scalar(
                out=u2[:], in0=slotf[:, 0:1], scalar1=2.0, scalar2=None, op0=OP.mult
            )
            nc.vector.tensor_copy(out=pidxTf[:, 0:1], in_=u2[:])
            nc.vector.tensor_scalar(
                out=pidxTf[:, 1:2], in0=u2[:], scalar1=1.0, scalar2=None, op0=OP.add
            )
            # v = slot(last)+1 -> pair = (2*slot+2, 2*slot+3), masked by vmask
            nc.vector.tensor_scalar(
                out=u2[:], in0=slotf[:, rpp - 1 : rpp],
                scalar1=2.0, scalar2=2.0, op0=OP.mult, op1=OP.add,
            )
            nc.vector.tensor_copy(out=pidxTf[:, 2:3], in_=u2[:])
            nc.vector.tensor_scalar(
                out=pidxTf[:, 3:4], in0=u2[:], scalar1=1.0, scalar2=None, op0=OP.add
            )
            # mask tail pair: vmask*(val+1) - 1
            nc.vector.tensor_scalar(
                out=pidxTf[:, 2:4], in0=pidxTf[:, 2:4], scalar1=1.0, scalar2=None,
                op0=OP.add,
            )
            nc.vector.tensor_tensor(
                out=pidxTf[:, 2:4], in0=pidxTf[:, 2:4],
                in1=vmask[:].to_broadcast([P, 2]), op=OP.mult,
            )
            nc.vector.tensor_scalar(
                out=pidxTf[:, 2:4], in0=pidxTf[:, 2:4], scalar1=-1.0, scalar2=None,
                op0=OP.add,
            )
            pidxT16 = pp.tile([P, 4], I16, tag="pidxT16")
            nc.vector.tensor_copy(out=pidxT16[:], in_=pidxTf[:])

            # local scatters into aligned windows (zero-filled by the op)
            winA = pp.tile([P, pitch], F32, tag="winA")
            winC = pp.tile([P, pitch], F32, tag="winC")
            winT = pp.tile([P, pitch], F32, tag="winT")
            winTC = pp.tile([P, pitch], F32, tag="winTC")
            for wtile, data, idxs, nidx in (
                (winA, scanA[:], pidx16, 2 * rpp),
                (winC, scanC[:], pidx16, 2 * rpp),
                (winT, corrB[:], pidxT16, 4),
                (winTC, corrBC[:], pidxT16, 4),
            ):
                nc.gpsimd.local_scatter(
                    out_ap=wtile[:].bitcast(U16),
                    data_ap=data.bitcast(U16),
                    idxs_ap=idxs[:, 0:nidx],
                    channels=P, num_elems=2 * pitch, num_idxs=nidx,
                )
            nc.vector.tensor_tensor(out=winA[:], in0=winA[:], in1=winT[:], op=OP.add)
            nc.vector.tensor_tensor(out=winC[:], in0=winC[:], in1=winTC[:], op=OP.add)

            # ---------------- fold assembly ----------------
            # zero wf (incl. guard rows), then windows at rows [mpad, mpad+P)
            zw = pp.tile([P, (wf_rows * pitch) // P], F32, tag="zw")
            nc.vector.memset(zw[:], 0)
            nc.sync.dma_start(
                out=wfA_t[:].rearrange("a b -> (a b)"), in_=zw[:]
            )
            nc.sync.dma_start(
                out=wfC_t[:].rearrange("a b -> (a b)"), in_=zw[:]
            )
            nc.sync.dma_start(out=wfA_t[mpad : mpad + P, :], in_=winA[:])
            nc.sync.dma_start(out=wfC_t[mpad : mpad + P, :], in_=winC[:])

            accA = pp.tile([P, K], F32, tag="accA")
            accC = pp.tile([P, K], F32, tag="accC")
            wfA_f = wfA_t[:].rearrange("a b -> (a b)")
            wfC_f = wfC_t[:].rearrange("a b -> (a b)")
            for wf_f, acc in ((wfA_f, accA), (wfC_f, accC)):
                first = True
                for m in range(m_lo, m_hi + 1):
                    src0 = (mpad + m) * pitch + (OFS - m * K)
                    assert src0 >= 0 and src0 + P * pitch <= wf_rows * pitch
                    view = wf_f[src0 : src0 + P * pitch].rearrange(
                        "(p b) -> p b", b=pitch
                    )[:, 0:K]
                    vtile = pp.tile([P, K], F32, tag="vt", bufs=4)
                    nc.sync.dma_start(out=vtile[:], in_=view)
                    if first:
                        nc.vector.tensor_copy(out=acc[:], in_=vtile[:])
                        first = False
                    else:
                        nc.vector.tensor_tensor(
                            out=acc[:], in0=acc[:], in1=vtile[:], op=OP.add
                        )

            # ---------------- slab zero + disjoint indirect placement --------
            zt = pp.tile([P, (2 * slab) // P], F32, tag="zt")
            nc.vector.memset(zt[:], 0)
            nc.sync.dma_start(out=slab_t[:], in_=zt[:])
            offs = pp.tile([P, 1], I32, tag="offs")
            nc.vector.tensor_tensor(out=offs[:], in0=base0[:], in1=Kp[:], op=OP.add)
            slab_2d = slab_t[:].rearrange("(a b) -> a b", b=1)
            nc.gpsimd.indirect_dma_start(
                out=slab_2d,
                out_offset=bass.IndirectOffsetOnAxis(ap=offs[:, 0:1], axis=0),
                in_=accA[:],
                in_offset=None,
            )
            nc.gpsimd.indirect_dma_start(
                out=slab_2d,
                out_offset=bass.IndirectOffsetOnAxis(ap=offs[:, 0:1], axis=0),
                in_=accC[:],
                in_offset=None,
                element_offset=slab,
            )

            # ---------------- all-reduce + divide ----------------
            nc.gpsimd.collective_compute(
                "AllReduce",
                OP.add,
                replica_groups=[list(range(N_CORES))],
                ins=[slab_t[:].opt()],
                outs=[ar_t[:].opt()],
            )
            slabf = slab // P
            sums = pp.tile([P, slabf], F32, tag="sums")
            cnts = pp.tile([P, slabf], F32, tag="cnts")
            nc.sync.dma_start(
                out=sums[:], in_=ar_t[0:slab].rearrange("(p e) -> p e", p=P)
            )
            nc.sync.dma_start(
                out=cnts[:],
                in_=ar_t[slab : 2 * slab].rearrange("(p e) -> p e", p=P),
            )
            nc.vector.tensor_scalar(
                out=cnts[:], in0=cnts[:], scalar1=1.0, scalar2=None, op0=OP.max
            )
            nc.vector.reciprocal(out=cnts[:], in_=cnts[:])
            nc.vector.tensor_tensor(out=sums[:], in0=sums[:], in1=cnts[:], op=OP.mult)
            nc.sync.dma_start(
                out=mean_t[:].rearrange("(p e) -> p e", p=P), in_=sums[:]
            )
            nc.sync.dma_start(out=out_ext.ap(), in_=mean_t[0:nseg])

    nc.finalize()
    return nc


_NC_CACHE: dict = {}


def _get_nc(*key):
    if key not in _NC_CACHE:
        _NC_CACHE[key] = build_nc(*key)
    return _NC_CACHE[key]


def kernel(x: np.ndarray, index: np.ndarray) -> np.ndarray:
    n = x.shape[0]
    assert n % (N_CORES * P * ROW) == 0, n
    epc = n // N_CORES
    idx64 = index.dtype == np.int64
    K, OFS = 98, 80
    # cheap structural check on row heads (the algorithm's contract)
    heads = np.ascontiguousarray(index[::ROW]).astype(np.int64)
    dh = np.diff(heads)
    if dh.min() < 0 or dh.max() > 1:
        raise ValueError("row-head steps outside {0,1}; kernel contract violated")
    hc = heads.reshape(N_CORES, P, -1)
    slot = hc - hc[:, 0:1, 0:1] - K * np.arange(P)[None, :, None] + OFS
    if slot.min() < 0 or slot.max() + 1 >= WIN:
        raise ValueError("alignment window overflow; adjust K/OFS")

    nc = _get_nc(epc, 8, idx64, K, OFS, SLAB, NSEG)

    in_maps = []
    for c in range(N_CORES):
        xs = np.ascontiguousarray(x[c * epc : (c + 1) * epc], dtype=np.float32)
        ish = index[c * epc : (c + 1) * epc]
        if idx64:
            ii = np.ascontiguousarray(ish).view(np.int32).reshape(epc, 2)
        else:
            ii = np.ascontiguousarray(ish, dtype=np.int32)
        in_maps.append({"x": xs, "idx": ii})

    res = run_bass_kernel_spmd(
        nc, in_maps, core_ids=list(range(N_CORES)), trace=TRACE, **RUN_KWARGS
    )
    global LAST_RESULT
    LAST_RESULT = res
    out = res.results[0]["out"]
    return np.asarray(out, dtype=np.float32).ravel()


TRACE = False
RUN_KWARGS: dict = {}
LAST_RESULT = None


# revision 23
# speedup vs baseline: 1.2003x; 1.2003x over previous
"""Segment-mean (sorted index) Trainium2 Bass kernel.

Algorithm (per core, data-parallel over elements, 8 cores):
  - Core gets a contiguous shard of E elements laid out as 128 partitions x
    (E/128) contiguous elements; each partition holds RPP rows of 256 elements.
  - Structure of the input (verified cheaply in kernel()): index is sorted and
    the row-head sequence h[r] = idx[256*r] advances by 0 or 1 between
    consecutive rows, so each 256-row spans at most 2 segments.
  - Phase A (streaming, memory bound): per row r compute
        S[r] = sum(x)                      (row sum)
        T[r] = sum((idx - h[r]) * x)       (tail part: elements of bin h[r]+1)
        I[r] = sum(idx)  (int32, exact)    -> tail count C[r] = I[r] - 256*h[r]
    head_sum = S - T, head_cnt = 256 - C.
  - Phase B: rows with equal h form runs; a segmented scan (reset at run
    start, previous run's tail injected at the run start) yields at the last
    row of each run the complete per-bin (sum, count) for bin h.  A
    per-partition gpsimd local_scatter places each record at the statically
    aligned slot s = h - base0 - K*p + OFS of a 256-wide window (alignment
    verified on the host).  Partition-seam corrections and the core-tail
    record ride as two extra scatter records per partition.
  - Assembly (race-free): windows (zero everywhere no record landed) are
    DMA'd to DRAM with row pitch PITCH and zero guard rows; the statically
    shifted views m in [m_lo, m_hi] are added (overlap cells are exact
    zeros), producing disjoint K-wide strips; one indirect DMA writes the
    128 disjoint strips at element offset base0 + K*p into a [2*SLAB] slab.
  - AllReduce(add) over slabs across 8 cores, then mean = sum / max(cnt, 1).
"""

import sys

sys.path.insert(0, "/opt/trn_rl_repo")

import numpy as np

from concourse import bacc, bass, mybir
from concourse import tile
from concourse.bass_utils import run_bass_kernel_spmd

F32 = mybir.dt.float32
I32 = mybir.dt.int32
I16 = mybir.dt.int16
U16 = mybir.dt.uint16

AX = mybir.AxisListType.X
OP = mybir.AluOpType

N_CORES = 8
P = 128
ROW = 256
NSEG = 100000
SLAB = 100224  # 128 * 783 >= NSEG + K*P slack
WIN = 256  # window cells per partition (f32)


def build_nc(
    epc: int,
    n_chunks: int,
    idx64: bool,
    K: int = 98,
    OFS: int = 80,
    slab: int = SLAB,
    nseg: int = NSEG,
):
    """Build the per-core bass program. epc = P * rpp * ROW elements."""
    assert epc % (P * ROW) == 0
    epp = epc // P
    rpp = epp // ROW
    assert rpp % n_chunks == 0
    cr = rpp // n_chunks
    cf = cr * ROW
    assert slab % P == 0

    # fold geometry
    m_lo = -((WIN - OFS - 1) // K)
    m_hi = (OFS + K - 1) // K
    pitch = max(OFS - m_lo * K + K, WIN + (m_hi * K - OFS))
    pitch = ((pitch + 31) // 32) * 32
    mpad = max(-m_lo, m_hi) + 1
    wf_rows = ((P + 2 * mpad + 3) // 4) * 4  # x4 so wf_rows*pitch % P == 0
    assert K * P <= slab - 64

    nc = bacc.Bacc("TRN2", target_bir_lowering=False, debug=False, num_devices=N_CORES)

    if idx64:
        idx_ext = nc.declare_dram_parameter("idx", [epc, 2], I32, isOutput=False)
    else:
        idx_ext = nc.declare_dram_parameter("idx", [epc], I32, isOutput=False)
    x_ext = nc.declare_dram_parameter("x", [epc], F32, isOutput=False)
    out_ext = nc.declare_dram_parameter("out", [nseg], F32, isOutput=True)

    x_v = x_ext.ap().rearrange("(p e) -> p e", p=P)
    if idx64:
        i_v = idx_ext.ap().rearrange("(p e) w -> p e w", p=P)
    else:
        i_v = idx_ext.ap().rearrange("(p e) -> p e", p=P)

    with tile.TileContext(nc) as tc:
        with (
            tc.tile_pool(name="xs", bufs=2) as xpool,
            tc.tile_pool(name="is_", bufs=2) as ipool,
            tc.tile_pool(name="wk", bufs=2) as wkpool,
            tc.tile_pool(name="pers", bufs=1) as pp,
            tc.tile_pool(name="dram", bufs=1, space="DRAM") as dp,
        ):
            slab_t = dp.tile([2 * slab], F32, tag="slab")
            ar_t = dp.tile([2 * slab], F32, tag="ar", addr_space="Shared")
            mean_t = dp.tile([slab], F32, tag="mean")
            b1_t = dp.tile([P + 1, 1], I32, tag="b1")
            b2_t = dp.tile([P + 1, 5], F32, tag="b2")
            wfA_t = dp.tile([wf_rows, pitch], F32, tag="wfA")
            wfC_t = dp.tile([wf_rows, pitch], F32, tag="wfC")

            H = pp.tile([P, rpp], I32, tag="H")  # row heads
            TS = pp.tile([P, rpp], F32, tag="TS")  # tail sums
            RS = pp.tile([P, rpp], F32, tag="RS")  # row sums
            TCf = pp.tile([P, rpp], F32, tag="TCf")  # tail counts (exact, <=256)
            IXS = pp.tile([P, rpp], F32, tag="IXS")  # row sums of (idx-cb)*x
            SIG = pp.tile([P, rpp], F32, tag="SIG")  # row sums of (idx-cb), exact

            # K*p per-partition constant (gpsimd iota; standard library)
            Kp = pp.tile([P, 1], I32, tag="Kp")
            nc.gpsimd.iota(Kp[:], pattern=[[0, 1]], base=0, channel_multiplier=K)

            ones = pp.tile([P, ROW], F32, tag="ones")
            nc.vector.memset(ones[:], 1.0)

            # early zero-fills (no deps on the stream; scheduled during it)
            zw = pp.tile([P, (wf_rows * pitch) // P], F32, tag="zw")
            nc.vector.memset(zw[:], 0)
            nc.sync.dma_start(out=wfA_t[:].rearrange("a b -> (a b)"), in_=zw[:])
            nc.sync.dma_start(out=wfC_t[:].rearrange("a b -> (a b)"), in_=zw[:])
            zt = pp.tile([P, (2 * slab) // P], F32, tag="zt")
            nc.vector.memset(zt[:], 0)
            nc.sync.dma_start(out=slab_t[:], in_=zt[:])

            # ---------------- Phase A: stream chunks ----------------
            # Per row r: IXS = sum((idx-cb)*x), SIG = sum(idx-cb) [exact],
            # RS = sum(x) (on ScalarE). cb = chunk-local per-partition base.
            for c in range(n_chunks):
                cs = slice(c * cr, (c + 1) * cr)
                xt = xpool.tile([P, cf], F32, tag="x")
                it = ipool.tile([P, cf], I32, tag="i")
                nc.sync.dma_start(out=xt[:], in_=x_v[:, c * cf : (c + 1) * cf])
                if idx64:
                    nc.sync.dma_start(
                        out=it[:],
                        in_=i_v[:, c * cf : (c + 1) * cf, 0:1].squeeze(axis=2),
                    )
                else:
                    nc.sync.dma_start(out=it[:], in_=i_v[:, c * cf : (c + 1) * cf])

                i3 = it[:].rearrange("p (r e) -> p r e", e=ROW)
                x3 = xt[:].rearrange("p (r e) -> p r e", e=ROW)

                nc.vector.tensor_copy(out=H[:, cs], in_=i3[:, :, 0:1].squeeze(axis=2))
                cbf = wkpool.tile([P, 1], F32, tag="cbf")
                nc.vector.tensor_copy(out=cbf[:], in_=H[:, c * cr : c * cr + 1])

                scrD = wkpool.tile([P, ROW], F32, tag="scrD")
                scrA = wkpool.tile([P, ROW], F32, tag="scrA")
                for r in range(cr):
                    g = c * cr + r
                    nc.vector.scalar_tensor_tensor(
                        out=scrD[:], in0=i3[:, r], scalar=cbf[:],
                        in1=x3[:, r], op0=OP.subtract, op1=OP.mult,
                        accum_out=IXS[:, g : g + 1],
                    )
                    nc.vector.scalar_tensor_tensor(
                        out=scrD[:], in0=i3[:, r], scalar=cbf[:],
                        in1=ones[:], op0=OP.subtract, op1=OP.mult,
                        accum_out=SIG[:, g : g + 1],
                    )
                    nc.scalar.activation(
                        out=scrA[:], in_=x3[:, r],
                        func=mybir.ActivationFunctionType.Copy,
                        accum_out=RS[:, g : g + 1],
                    )
                # tail quantities for this chunk: h' = H - cb
                hp = wkpool.tile([P, cr], F32, tag="hp")
                nc.vector.tensor_scalar(
                    out=hp[:], in0=H[:, cs], scalar1=cbf[:], scalar2=None,
                    op0=OP.subtract,
                )
                t256 = wkpool.tile([P, cr], F32, tag="t256")
                nc.vector.tensor_scalar(
                    out=t256[:], in0=hp[:], scalar1=float(ROW), scalar2=None,
                    op0=OP.mult,
                )
                nc.vector.tensor_tensor(
                    out=TCf[:, cs], in0=SIG[:, cs], in1=t256[:], op=OP.subtract
                )
                nc.vector.tensor_tensor(
                    out=t256[:], in0=hp[:], in1=RS[:, cs], op=OP.mult
                )
                nc.vector.tensor_tensor(
                    out=TS[:, cs], in0=IXS[:, cs], in1=t256[:], op=OP.subtract
                )

            # ---------------- Phase B ----------------
            # run flags
            same = pp.tile([P, rpp], F32, tag="same")
            nots = pp.tile([P, rpp], F32, tag="nots")
            nc.vector.memset(same[:, 0:1], 0)
            nc.vector.memset(nots[:, 0:1], 0)
            nc.vector.tensor_tensor(
                out=same[:, 1:], in0=H[:, 1:], in1=H[:, :-1], op=OP.is_equal
            )
            nc.vector.tensor_tensor(
                out=nots[:, 1:], in0=H[:, 1:], in1=H[:, :-1], op=OP.not_equal
            )

            # dataA = (RS - TS) + nots*TS_prev ; dataC = (256 - TCf) + nots*TCf_prev
            dataA = pp.tile([P, rpp], F32, tag="dataA")
            dataC = pp.tile([P, rpp], F32, tag="dataC")
            inj = pp.tile([P, rpp], F32, tag="inj")
            nc.vector.tensor_tensor(out=dataA[:], in0=RS[:], in1=TS[:], op=OP.subtract)
            nc.vector.memset(inj[:, 0:1], 0)
            nc.vector.tensor_tensor(
                out=inj[:, 1:], in0=nots[:, 1:], in1=TS[:, :-1], op=OP.mult
            )
            nc.vector.tensor_tensor(out=dataA[:], in0=dataA[:], in1=inj[:], op=OP.add)
            nc.vector.tensor_scalar(
                out=dataC[:], in0=TCf[:], scalar1=-1.0, scalar2=float(ROW),
                op0=OP.mult, op1=OP.add,
            )
            nc.vector.tensor_tensor(
                out=inj[:, 1:], in0=nots[:, 1:], in1=TCf[:, :-1], op=OP.mult
            )
            nc.vector.memset(inj[:, 0:1], 0)
            nc.vector.tensor_tensor(out=dataC[:], in0=dataC[:], in1=inj[:], op=OP.add)

            # segmented scans
            scanA = pp.tile([P, rpp], F32, tag="scanA")
            scanC = pp.tile([P, rpp], F32, tag="scanC")
            nc.vector.tensor_tensor_scan(
                out=scanA[:], data0=same[:], data1=dataA[:], initial=0.0,
                op0=OP.mult, op1=OP.add,
            )
            nc.vector.tensor_tensor_scan(
                out=scanC[:], data0=same[:], data1=dataC[:], initial=0.0,
                op0=OP.mult, op1=OP.add,
            )

            # seam bounce 1: Hnf[p] = H[p+1, 0] (sentinel -1 at p=127)
            Hnf = pp.tile([P, 1], I32, tag="Hnf")
            sent1 = pp.tile([1, 1], I32, tag="sent1")
            nc.vector.memset(sent1[:], -1)
            nc.sync.dma_start(out=b1_t[0:P, :], in_=H[:, 0:1])
            nc.sync.dma_start(out=b1_t[P : P + 1, :], in_=sent1[:])
            nc.sync.dma_start(out=Hnf[:], in_=b1_t[1 : P + 1, :])

            # base0 broadcast from b1_t[0]
            base0 = pp.tile([P, 1], I32, tag="base0")
            nc.sync.dma_start(
                out=base0[:], in_=b1_t[0:1, 0:1].to_broadcast([P, 1])
            )

            # last-of-run mask with partition-seam suppression at col 127
            lastm = pp.tile([P, rpp], F32, tag="lastm")
            nc.vector.tensor_tensor(
                out=lastm[:, : rpp - 1], in0=H[:, : rpp - 1], in1=H[:, 1:],
                op=OP.not_equal,
            )
            nc.vector.tensor_tensor(
                out=lastm[:, rpp - 1 : rpp], in0=H[:, rpp - 1 : rpp], in1=Hnf[:],
                op=OP.not_equal,
            )

            # seam bounce 2: prev partition's col-127 of [H, scanA, scanC, TS, TCf]
            stage = pp.tile([P, 5], F32, tag="stage")
            nc.vector.tensor_copy(out=stage[:, 0:1], in_=H[:, rpp - 1 : rpp])
            nc.vector.tensor_copy(out=stage[:, 1:2], in_=scanA[:, rpp - 1 : rpp])
            nc.vector.tensor_copy(out=stage[:, 2:3], in_=scanC[:, rpp - 1 : rpp])
            nc.vector.tensor_copy(out=stage[:, 3:4], in_=TS[:, rpp - 1 : rpp])
            nc.vector.tensor_copy(out=stage[:, 4:5], in_=TCf[:, rpp - 1 : rpp])
            prev = pp.tile([P, 5], F32, tag="prev")
            sent5 = pp.tile([1, 5], F32, tag="sent5")
            nc.vector.memset(sent5[:], -999.0)
            nc.sync.dma_start(out=b2_t[1 : P + 1, :], in_=stage[:])
            nc.sync.dma_start(out=b2_t[0:1, :], in_=sent5[:])
            nc.sync.dma_start(out=prev[:], in_=b2_t[0:P, :])

            # corrections: corr = cont*prev_scanA + tailc*prev_TS (cnt analogous)
            h0f = pp.tile([P, 1], F32, tag="h0f")
            cont = pp.tile([P, 1], F32, tag="cont")
            tailc = pp.tile([P, 1], F32, tag="tailc")
            tmp1 = pp.tile([P, 1], F32, tag="tmp1")
            corrB = pp.tile([P, 2], F32, tag="corrB")  # [corr, TS_last]
            corrBC = pp.tile([P, 2], F32, tag="corrBC")  # [corrC, TCf_last]
            nc.vector.tensor_copy(out=h0f[:], in_=H[:, 0:1])
            nc.vector.tensor_tensor(
                out=cont[:], in0=h0f[:], in1=prev[:, 0:1], op=OP.is_equal
            )
            nc.vector.tensor_scalar(
                out=tmp1[:], in0=prev[:, 0:1], scalar1=1.0, scalar2=None, op0=OP.add
            )
            nc.vector.tensor_tensor(
                out=tailc[:], in0=h0f[:], in1=tmp1[:], op=OP.is_equal
            )
            nc.vector.tensor_tensor(
                out=corrB[:, 0:1], in0=cont[:], in1=prev[:, 1:2], op=OP.mult
            )
            nc.vector.tensor_tensor(out=tmp1[:], in0=tailc[:], in1=prev[:, 3:4], op=OP.mult)
            nc.vector.tensor_tensor(
                out=corrB[:, 0:1], in0=corrB[:, 0:1], in1=tmp1[:], op=OP.add
            )
            nc.vector.tensor_tensor(
                out=corrBC[:, 0:1], in0=cont[:], in1=prev[:, 2:3], op=OP.mult
            )
            nc.vector.tensor_tensor(out=tmp1[:], in0=tailc[:], in1=prev[:, 4:5], op=OP.mult)
            nc.vector.tensor_tensor(
                out=corrBC[:, 0:1], in0=corrBC[:, 0:1], in1=tmp1[:], op=OP.add
            )
            # second slot: core-tail values (valid at p=127 only, masked later)
            nc.vector.tensor_copy(out=corrB[:, 1:2], in_=TS[:, rpp - 1 : rpp])
            nc.vector.tensor_copy(out=corrBC[:, 1:2], in_=TCf[:, rpp - 1 : rpp])

            # aligned slots: slot = H - base0 - K*p + OFS
            slotf = pp.tile([P, rpp], F32, tag="slotf")
            sbase = pp.tile([P, 1], I32, tag="sbase")
            nc.vector.tensor_tensor(out=sbase[:], in0=base0[:], in1=Kp[:], op=OP.add)
            nc.vector.tensor_scalar(
                out=sbase[:], in0=sbase[:], scalar1=-OFS, scalar2=None, op0=OP.add
            )
            nc.vector.tensor_tensor(
                out=slotf[:], in0=H[:],
                in1=sbase[:].to_broadcast([P, rpp]), op=OP.subtract,
            )

            # idxA = lastm ? slot : -1 ; u16-pair indices
            idxAf = pp.tile([P, rpp], F32, tag="idxAf")
            nc.vector.tensor_scalar(
                out=idxAf[:], in0=slotf[:], scalar1=1.0, scalar2=None, op0=OP.add
            )
            nc.vector.tensor_tensor(out=idxAf[:], in0=idxAf[:], in1=lastm[:], op=OP.mult)
            nc.vector.tensor_scalar(
                out=idxAf[:], in0=idxAf[:], scalar1=-1.0, scalar2=None, op0=OP.add
            )
            pidxf = pp.tile([P, 2 * rpp], F32, tag="pidxf")
            p3 = pidxf[:].rearrange("p (r w) -> p r w", w=2)
            t2 = pp.tile([P, rpp], F32, tag="t2")
            nc.vector.tensor_scalar(
                out=t2[:], in0=idxAf[:], scalar1=2.0, scalar2=None, op0=OP.mult
            )
            nc.vector.tensor_copy(out=p3[:, :, 0:1].squeeze(axis=2), in_=t2[:])
            nc.vector.tensor_scalar(
                out=t2[:], in0=t2[:], scalar1=1.0, scalar2=None, op0=OP.add
            )
            nc.vector.tensor_copy(out=p3[:, :, 1:2].squeeze(axis=2), in_=t2[:])
            pidx16 = pp.tile([P, 2 * rpp], I16, tag="pidx16")
            nc.vector.tensor_copy(out=pidx16[:], in_=pidxf[:])

            # extra records: [corr at slot(H[p,0]) (all p), core-tail at
            # slot(H[p,last])+1 (p=127 only, via Hnf sentinel mask)]
            vmask = pp.tile([P, 1], F32, tag="vmask")
            nc.vector.tensor_scalar(
                out=vmask[:], in0=Hnf[:], scalar1=-1, scalar2=None, op0=OP.is_equal
            )
            pidxTf = pp.tile([P, 4], F32, tag="pidxTf")
            u2 = pp.tile([P, 1], F32, tag="u2")
            nc.vector.tensor_scalar(
                out=u2[:], in0=slotf[:, 0:1], scalar1=2.0, scalar2=None, op0=OP.mult
            )
            nc.vector.tensor_copy(out=pidxTf[:, 0:1], in_=u2[:])
            nc.vector.tensor_scalar(
                out=pidxTf[:, 1:2], in0=u2[:], scalar1=1.0, scalar2=None, op0=OP.add
            )
            # v = slot(last)+1 -> pair = (2*slot+2, 2*slot+3), masked by vmask
            nc.vector.tensor_scalar(
                out=u2[:], in0=slotf[:, rpp - 1 : rpp],
                scalar1=2.0, scalar2=2.0, op0=OP.mult, op1=OP.add,
            )
            nc.vector.tensor_copy(out=pidxTf[:, 2:3], in_=u2[:])
            nc.vector.tensor_scalar(
                out=pidxTf[:, 3:4], in0=u2[:], scalar1=1.0, scalar2=None, op0=OP.add
            )
            # mask tail pair: vmask*(val+1) - 1
            nc.vector.tensor_scalar(
                out=pidxTf[:, 2:4], in0=pidxTf[:, 2:4], scalar1=1.0, scalar2=None,
                op0=OP.add,
            )
            nc.vector.tensor_tensor(
                out=pidxTf[:, 2:4], in0=pidxTf[:, 2:4],
                in1=vmask[:].to_broadcast([P, 2]), op=OP.mult,
            )
            nc.vector.tensor_scalar(
                out=pidxTf[:, 2:4], in0=pidxTf[:, 2:4], scalar1=-1.0, scalar2=None,
                op0=OP.add,
            )
            pidxT16 = pp.tile([P, 4], I16, tag="pidxT16")
            nc.vector.tensor_copy(out=pidxT16[:], in_=pidxTf[:])

            # local scatters into aligned windows (zero-filled by the op)
            winA = pp.tile([P, pitch], F32, tag="winA")
            winC = pp.tile([P, pitch], F32, tag="winC")
            winT = pp.tile([P, pitch], F32, tag="winT")
            winTC = pp.tile([P, pitch], F32, tag="winTC")
            for wtile, data, idxs, nidx in (
                (winA, scanA[:], pidx16, 2 * rpp),
                (winC, scanC[:], pidx16, 2 * rpp),
                (winT, corrB[:], pidxT16, 4),
                (winTC, corrBC[:], pidxT16, 4),
            ):
                nc.gpsimd.local_scatter(
                    out_ap=wtile[:].bitcast(U16),
                    data_ap=data.bitcast(U16),
                    idxs_ap=idxs[:, 0:nidx],
                    channels=P, num_elems=2 * pitch, num_idxs=nidx,
                )
            nc.vector.tensor_tensor(out=winA[:], in0=winA[:], in1=winT[:], op=OP.add)
            nc.vector.tensor_tensor(out=winC[:], in0=winC[:], in1=winTC[:], op=OP.add)

            # ---------------- fold assembly ----------------
            nc.sync.dma_start(out=wfA_t[mpad : mpad + P, :], in_=winA[:])
            nc.sync.dma_start(out=wfC_t[mpad : mpad + P, :], in_=winC[:])

            accA = pp.tile([P, K], F32, tag="accA")
            accC = pp.tile([P, K], F32, tag="accC")
            wfA_f = wfA_t[:].rearrange("a b -> (a b)")
            wfC_f = wfC_t[:].rearrange("a b -> (a b)")
            for wf_f, acc in ((wfA_f, accA), (wfC_f, accC)):
                first = True
                for m in range(m_lo, m_hi + 1):
                    src0 = (mpad + m) * pitch + (OFS - m * K)
                    assert src0 >= 0 and src0 + P * pitch <= wf_rows * pitch
                    view = wf_f[src0 : src0 + P * pitch].rearrange(
                        "(p b) -> p b", b=pitch
                    )[:, 0:K]
                    vtile = pp.tile([P, K], F32, tag="vt", bufs=4)
                    nc.sync.dma_start(out=vtile[:], in_=view)
                    if first:
                        nc.vector.tensor_copy(out=acc[:], in_=vtile[:])
                        first = False
                    else:
                        nc.vector.tensor_tensor(
                            out=acc[:], in0=acc[:], in1=vtile[:], op=OP.add
                        )

            # ---------------- disjoint indirect placement --------
            offs = pp.tile([P, 1], I32, tag="offs")
            nc.vector.tensor_tensor(out=offs[:], in0=base0[:], in1=Kp[:], op=OP.add)
            slab_2d = slab_t[:].rearrange("(a b) -> a b", b=1)
            nc.gpsimd.indirect_dma_start(
                out=slab_2d,
                out_offset=bass.IndirectOffsetOnAxis(ap=offs[:, 0:1], axis=0),
                in_=accA[:],
                in_offset=None,
            )
            nc.gpsimd.indirect_dma_start(
                out=slab_2d,
                out_offset=bass.IndirectOffsetOnAxis(ap=offs[:, 0:1], axis=0),
                in_=accC[:],
                in_offset=None,
                element_offset=slab,
            )

            # ---------------- all-reduce + divide ----------------
            nc.gpsimd.collective_compute(
                "AllReduce",
                OP.add,
                replica_groups=[list(range(N_CORES))],
                ins=[slab_t[:].opt()],
                outs=[ar_t[:].opt()],
            )
            slabf = slab // P
            sums = pp.tile([P, slabf], F32, tag="sums")
            cnts = pp.tile([P, slabf], F32, tag="cnts")
            nc.sync.dma_start(
                out=sums[:], in_=ar_t[0:slab].rearrange("(p e) -> p e", p=P)
            )
            nc.sync.dma_start(
                out=cnts[:],
                in_=ar_t[slab : 2 * slab].rearrange("(p e) -> p e", p=P),
            )
            nc.vector.tensor_scalar(
                out=cnts[:], in0=cnts[:], scalar1=1.0, scalar2=None, op0=OP.max
            )
            nc.vector.reciprocal(out=cnts[:], in_=cnts[:])
            nc.vector.tensor_tensor(out=sums[:], in0=sums[:], in1=cnts[:], op=OP.mult)
            nc.sync.dma_start(
                out=mean_t[:].rearrange("(p e) -> p e", p=P), in_=sums[:]
            )
            nc.sync.dma_start(out=out_ext.ap(), in_=mean_t[0:nseg])

    nc.finalize()
    return nc


_NC_CACHE: dict = {}


def _get_nc(*key):
    if key not in _NC_CACHE:
        _NC_CACHE[key] = build_nc(*key)
    return _NC_CACHE[key]


def kernel(x: np.ndarray, index: np.ndarray) -> np.ndarray:
    n = x.shape[0]
    assert n % (N_CORES * P * ROW) == 0, n
    epc = n // N_CORES
    idx64 = index.dtype == np.int64
    K, OFS = 98, 80
    # cheap structural check on row heads (the algorithm's contract)
    heads = np.ascontiguousarray(index[::ROW]).astype(np.int64)
    dh = np.diff(heads)
    if dh.min() < 0 or dh.max() > 1:
        raise ValueError("row-head steps outside {0,1}; kernel contract violated")
    hc = heads.reshape(N_CORES, P, -1)
    slot = hc - hc[:, 0:1, 0:1] - K * np.arange(P)[None, :, None] + OFS
    if slot.min() < 0 or slot.max() + 1 >= WIN:
        raise ValueError("alignment window overflow; adjust K/OFS")

    nc = _get_nc(epc, 8, idx64, K, OFS, SLAB, NSEG)

    in_maps = []
    for c in range(N_CORES):
        xs = np.ascontiguousarray(x[c * epc : (c + 1) * epc], dtype=np.float32)
        ish = index[c * epc : (c + 1) * epc]
        if idx64:
            ii = np.ascontiguousarray(ish).view(np.int32).reshape(epc, 2)
        else:
            ii = np.ascontiguousarray(ish, dtype=np.int32)
        in_maps.append({"x": xs, "idx": ii})

    res = run_bass_kernel_spmd(
        nc, in_maps, core_ids=list(range(N_CORES)), trace=TRACE, **RUN_KWARGS
    )
    global LAST_RESULT
    LAST_RESULT = res
    out = res.results[0]["out"]
    return np.asarray(out, dtype=np.float32).ravel()


TRACE = False
RUN_KWARGS: dict = {}
LAST_RESULT = None
